# revision 28
# baseline (speedup 1.0000x reference)
"""Trainium2 Bass kernel for the FFSNN v2 problem.

Math (per timestep t, reference semantics):
    m_l = m_l * 0.2 * (1 - s_l) + inp @ W_l.T + b_l ;  s_l = (m_l > 0.5)
    acc += s3 @ W4.T + b4 ;  out = acc/T                      (T = 196)

Device formulation used here (verified vs fp32 reference offline,
absmax err ~2e-6):
  * "not-spike" ns = (m <= 0.5) replaces s: W @ s = rowsum(W) - W @ ns,
    so the per-layer bias becomes b' = b + rowsum(W) and the matmul input
    is ns, which also serves as the decay mask: m' = (0.2*m)*ns_prev + b' - W@ns.
  * W2/W3/W4 are split hi/lo into two bf16 matrices (W ≈ hi + lo); ns is
    exact {0,1} in bf16, so 2 bf16 matmuls + fp32 PSUM accumulate give
    ~fp32 matmul accuracy at 2x bf16 cost (vs 4x for native fp32).
  * Layer 1 (K=4, tiny) is hoisted out of the time loop: only 49 distinct
    input slices exist (the reference indexing quirk reuses the last 4
    columns for t>=48). delta1[idx] = x_slice @ W1.T + b1 is precomputed
    on device with 6-pass bf16 (x and W1 each split into 3 exact bf16
    parts; terms with combined order <= 2 kept) at N=512 — layer 1 sees
    the same input every step, so its delta must be fp32-exact or the
    persistent residual flips steady-state neurons (measured 2e-3 with a
    2-part split vs 2e-6 with this 3-part split).
  * Output acc is accumulated as sum_t ns3 (bf16 integers, exact), and the
    single W4 matmul + affine happens once at the end:
    out = (b4 + rowsum(W4)) - (sum_t ns3) @ W4.T / T.

Layout: batch shard of 128 per core on the matmul free dim; hidden (512)
as 4 chunks of 128 on partitions. State tiles are [128, 512] with free
index = chunk*128 + batch, which makes chunk k of a state tile directly
usable as the [K=128, N=128] moving operand of the next layer's matmuls.

Sharding: pure data parallel over 8 NeuronCores (batch 1024 -> 128/core),
weights replicated, no collectives.
"""

import os
import sys

sys.path.insert(0, "/opt/trn_rl_repo")

import numpy as np
import ml_dtypes

BF16 = ml_dtypes.bfloat16
F32 = np.float32

NCORES = 8
B = 1024
BC = B // NCORES          # 128 batch per core
T = 196
H = 512
NJ = 4                    # hidden chunks of 128
NIDX = 49                 # distinct layer-1 input slices
XG_N = NIDX * BC          # 6272
PRE_BLKS = 13             # ceil(6272/512); last block is 128 wide

_BUILT = None             # (nc, input_names)
LAST_EXEC_NS = None
LAST_RESULTS = None
_REPEAT = 1               # benchmarking knob: run the time loop N times


def _split(a):
    hi = a.astype(BF16)
    lo = (a.astype(F32) - hi.astype(F32)).astype(BF16)
    return hi, lo


def _split3(a):
    """Exact 3-way bf16 decomposition (24 mantissa bits = 3x8)."""
    p0 = a.astype(BF16)
    r = a.astype(F32) - p0.astype(F32)
    p1 = r.astype(BF16)
    p2 = (r - p1.astype(F32)).astype(BF16)
    return p0, p1, p2


def _prep_host(inputs):
    x = np.ascontiguousarray(inputs["x"], dtype=F32)          # [1024, 784]
    W1 = np.asarray(inputs["W1"], F32); b1 = np.asarray(inputs["b1"], F32)
    W2 = np.asarray(inputs["W2"], F32); b2 = np.asarray(inputs["b2"], F32)
    W3 = np.asarray(inputs["W3"], F32); b3 = np.asarray(inputs["b3"], F32)
    W4 = np.asarray(inputs["W4"], F32); b4 = np.asarray(inputs["b4"], F32)

    # gathered input, aug with ones row: xg[c, k, idx*BC + b] = x[c*BC+b, base+k]
    bases = [4 * i for i in range(48)] + [780]
    xg = np.empty((NCORES, 5, XG_N), F32)
    for i, s in enumerate(bases):
        blk = x[:, s:s + 4].reshape(NCORES, BC, 4).transpose(0, 2, 1)
        xg[:, 0:4, i * BC:(i + 1) * BC] = blk
    xg[:, 4, :] = 1.0
    x0, x1, x2 = _split3(xg)                                   # [8, 5, 6272]

    W1a = np.concatenate([W1.T, b1[None, :]], axis=0)          # [5, 512]
    w1p0, w1p1, w1p2 = _split3(W1a)

    def wlay(W):  # [512,512] -> two bf16 [128, 2048] in (k,j)-block layout
        WT = W.T.copy()
        hi, lo = _split(WT)
        def lay(a):
            return np.ascontiguousarray(
                a.reshape(4, 128, 4, 128).transpose(1, 0, 2, 3).reshape(128, 2048))
        return lay(hi), lay(lo)

    w2h, w2l = wlay(W2)
    w3h, w3l = wlay(W3)

    W4T = W4.T.copy()                                          # [512, 10]
    h4, l4 = _split(W4T)
    def lay4(a):
        return np.ascontiguousarray(a.reshape(4, 128, 10).transpose(1, 0, 2).reshape(128, 40))
    w4h, w4l = lay4(h4), lay4(l4)

    f64 = np.float64
    b2p = (b2.astype(f64) + W2.astype(f64).sum(1)).astype(F32)
    b3p = (b3.astype(f64) + W3.astype(f64).sum(1)).astype(F32)
    outb = (b4.astype(f64) + W4.astype(f64).sum(1)).astype(F32).reshape(10, 1)
    b2c = np.ascontiguousarray(b2p.reshape(4, 128).T)          # [128, 4]
    b3c = np.ascontiguousarray(b3p.reshape(4, 128).T)

    shared = dict(w1p0=w1p0, w1p1=w1p1, w1p2=w1p2, w2h=w2h, w2l=w2l,
                  w3h=w3h, w3l=w3l, w4h=w4h, w4l=w4l, b2c=b2c, b3c=b3c,
                  outb=outb)
    in_maps = []
    for c in range(NCORES):
        m = dict(shared)
        m["x0"] = np.ascontiguousarray(x0[c])
        m["x1"] = np.ascontiguousarray(x1[c])
        m["x2"] = np.ascontiguousarray(x2[c])
        in_maps.append(m)
    return in_maps


def _build():
    import concourse.mybir as mybir
    import concourse.tile as tile
    from concourse import bacc

    dt = mybir.dt
    op = mybir.AluOpType
    AF = mybir.ActivationFunctionType

    nc = bacc.Bacc()

    d_x0 = nc.dram_tensor("x0", [5, XG_N], dt.bfloat16, kind="ExternalInput")
    d_x1 = nc.dram_tensor("x1", [5, XG_N], dt.bfloat16, kind="ExternalInput")
    d_x2 = nc.dram_tensor("x2", [5, XG_N], dt.bfloat16, kind="ExternalInput")
    d_w1p0 = nc.dram_tensor("w1p0", [5, H], dt.bfloat16, kind="ExternalInput")
    d_w1p1 = nc.dram_tensor("w1p1", [5, H], dt.bfloat16, kind="ExternalInput")
    d_w1p2 = nc.dram_tensor("w1p2", [5, H], dt.bfloat16, kind="ExternalInput")
    d_w2h = nc.dram_tensor("w2h", [128, 2048], dt.bfloat16, kind="ExternalInput")
    d_w2l = nc.dram_tensor("w2l", [128, 2048], dt.bfloat16, kind="ExternalInput")
    d_w3h = nc.dram_tensor("w3h", [128, 2048], dt.bfloat16, kind="ExternalInput")
    d_w3l = nc.dram_tensor("w3l", [128, 2048], dt.bfloat16, kind="ExternalInput")
    d_w4h = nc.dram_tensor("w4h", [128, 40], dt.bfloat16, kind="ExternalInput")
    d_w4l = nc.dram_tensor("w4l", [128, 40], dt.bfloat16, kind="ExternalInput")
    d_b2c = nc.dram_tensor("b2c", [128, 4], dt.float32, kind="ExternalInput")
    d_b3c = nc.dram_tensor("b3c", [128, 4], dt.float32, kind="ExternalInput")
    d_outb = nc.dram_tensor("outb", [10, 1], dt.float32, kind="ExternalInput")
    d_y = nc.dram_tensor("y", [10, BC], dt.float32, kind="ExternalOutput")

    with tile.TileContext(nc) as tc:
        with tc.tile_pool(name="const", bufs=1) as cp:
            x0 = cp.tile([5, XG_N], dt.bfloat16)
            x1 = cp.tile([5, XG_N], dt.bfloat16)
            x2 = cp.tile([5, XG_N], dt.bfloat16)
            w1p0 = cp.tile([5, H], dt.bfloat16)
            w1p1 = cp.tile([5, H], dt.bfloat16)
            w1p2 = cp.tile([5, H], dt.bfloat16)
            w2h = cp.tile([128, 2048], dt.bfloat16)
            w2l = cp.tile([128, 2048], dt.bfloat16)
            w3h = cp.tile([128, 2048], dt.bfloat16)
            w3l = cp.tile([128, 2048], dt.bfloat16)
            w4h = cp.tile([128, 40], dt.bfloat16)
            w4l = cp.tile([128, 40], dt.bfloat16)
            b2c = cp.tile([128, 4], dt.float32)
            b3c = cp.tile([128, 4], dt.float32)
            outb = cp.tile([10, 1], dt.float32)
            for sb, dr in [(x0, d_x0), (x1, d_x1), (x2, d_x2), (w1p0, d_w1p0),
                           (w1p1, d_w1p1), (w1p2, d_w1p2),
                           (w2h, d_w2h), (w2l, d_w2l), (w3h, d_w3h), (w3l, d_w3l),
                           (w4h, d_w4h), (w4l, d_w4l), (b2c, d_b2c), (b3c, d_b3c),
                           (outb, d_outb)]:
                nc.sync.dma_start(sb, dr[:])

            delta1 = cp.tile([128, NIDX * H], dt.float32)      # 98 KB/partition
            mem1 = cp.tile([128, H], dt.float32)
            mem2 = cp.tile([128, H], dt.float32)
            mem3 = cp.tile([128, H], dt.float32)
            ns1 = cp.tile([128, H], dt.bfloat16)
            ns2 = cp.tile([128, H], dt.bfloat16)
            ns3 = cp.tile([128, H], dt.bfloat16)
            u1 = cp.tile([128, H], dt.float32)
            u2 = cp.tile([128, H], dt.float32)
            u3 = cp.tile([128, H], dt.float32)
            accns = cp.tile([128, H], dt.bfloat16)
            y_sb = cp.tile([10, BC], dt.float32)

            nc.vector.memset(mem1, 0.0)
            nc.vector.memset(mem2, 0.0)
            nc.vector.memset(mem3, 0.0)
            nc.vector.memset(ns1, 1.0)
            nc.vector.memset(ns2, 1.0)
            nc.vector.memset(ns3, 1.0)
            nc.vector.memset(accns, 0.0)

            # ---- layer-1 precompute: delta1[idx] = x_slice @ W1.T + b1 ----
            # 4-pass bf16 (hi/lo x  X  hi/lo W1), N=512 moving blocks.
            d1v = delta1.rearrange("p (i j b) -> p i j b", j=NJ, b=BC)
            with tc.tile_pool(name="ppre", bufs=4, space="PSUM") as ppre:
                for j in range(NJ):
                    for blk in range(PRE_BLKS):
                        n = 512 if blk < PRE_BLKS - 1 else XG_N - 512 * (PRE_BLKS - 1)
                        nq = n // BC
                        ps = ppre.tile([128, 512], dt.float32, tag="pre")
                        # 6-pass exact-ish product: terms (i,j) with i+j<=2
                        # of x = x0+x1+x2, W1a = w0+w1+w2 (residual ~2^-26)
                        passes = [(w1p0, x0), (w1p0, x1), (w1p1, x0),
                                  (w1p1, x1), (w1p0, x2), (w1p2, x0)]
                        for pi, (wa, xa) in enumerate(passes):
                            nc.tensor.matmul(
                                ps[:, :n],
                                wa[:, j * 128:(j + 1) * 128],
                                xa[:, blk * 512:blk * 512 + n],
                                start=(pi == 0), stop=(pi == len(passes) - 1))
                        src = ps.rearrange("p (q b) -> p q b", b=BC)[:, :nq, :]
                        dst = d1v[:, 4 * blk:4 * blk + nq, j, :]
                        nc.scalar.activation(dst, src, AF.Copy)

            # ---- time loop (fully unrolled) ----
            with tc.tile_pool(name="pmm", bufs=1, space="PSUM") as pp:
                halves2 = (w2h, w2l)
                halves3 = (w3h, w3l)
                for t in range(T * _REPEAT):
                    idx = min(t % T, 48)

                    # layer 1 (DVE): m1 = (0.2*m1)*ns1 + delta1[idx]
                    # (gpsimd measured far slower than DVE for these shapes)
                    nc.vector.scalar_tensor_tensor(
                        u1, mem1, 0.2, ns1, op.mult, op.mult)
                    nc.vector.tensor_tensor(
                        mem1, u1, delta1[:, idx * H:(idx + 1) * H], op.add)
                    nc.vector.tensor_scalar(ns1, mem1, 0.5, None, op.is_le)

                    # decayed-masked state for layers 2/3 (reads t-1 state)
                    nc.vector.scalar_tensor_tensor(
                        u2, mem2, 0.2, ns2, op.mult, op.mult)
                    nc.vector.scalar_tensor_tensor(
                        u3, mem3, 0.2, ns3, op.mult, op.mult)

                    # layer 2 matmuls: P2_j = (W2 @ ns1) chunk j (hi+lo).
                    # One psum BANK per output chunk: PE-write + DVE-read of
                    # the same bank is a HW hazard Tile serializes, so chunks
                    # in separate banks let chunk j's membrane update overlap
                    # chunk j+1's matmuls. start=True clears its whole bank.
                    for j in range(NJ):
                        P2j = pp.tile([128, 128], dt.float32, tag=f"P2_{j}")
                        for k in range(NJ):
                            for h, wt in enumerate(halves2):
                                nc.tensor.matmul(
                                    P2j,
                                    wt[:, (k * NJ + j) * 128:(k * NJ + j + 1) * 128],
                                    ns1[:, k * 128:(k + 1) * 128],
                                    start=(k == 0 and h == 0),
                                    stop=(k == NJ - 1 and h == 1))
                        # m2_j = u2_j + b2'_j - P2_j ; ns2_j = (m2_j <= 0.5)
                        js = slice(j * 128, (j + 1) * 128)
                        nc.vector.scalar_tensor_tensor(
                            mem2[:, js], u2[:, js], b2c[:, j:j + 1], P2j,
                            op.add, op.subtract)
                        nc.vector.tensor_scalar(
                            ns2[:, js], mem2[:, js], 0.5, None, op.is_le)

                    # layer 3 matmuls: P3_j = (W3 @ ns2) chunk j (hi+lo).
                    # k-OUTER: the k=0 pass depends only on ns2 chunk 0,
                    # which is ready while layer-2 matmuls still run -> no
                    # PE gap between the layers.
                    P3t = [pp.tile([128, 128], dt.float32, tag=f"P3_{j}",
                                   name=f"P3_{j}_{t}") for j in range(NJ)]
                    for k in range(NJ):
                        for h, wt in enumerate(halves3):
                            for j in range(NJ):
                                nc.tensor.matmul(
                                    P3t[j],
                                    wt[:, (k * NJ + j) * 128:(k * NJ + j + 1) * 128],
                                    ns2[:, k * 128:(k + 1) * 128],
                                    start=(k == 0 and h == 0),
                                    stop=(k == NJ - 1 and h == 1))
                    for j in range(NJ):
                        js = slice(j * 128, (j + 1) * 128)
                        nc.vector.scalar_tensor_tensor(
                            mem3[:, js], u3[:, js], b3c[:, j:j + 1], P3t[j],
                            op.add, op.subtract)
                    nc.vector.tensor_scalar(ns3, mem3, 0.5, None, op.is_le)
                    nc.vector.tensor_tensor(accns, accns, ns3, op.add)

                # ---- readout: y = outb - (W4 @ accns)/T ----
                Pout = pp.tile([10, BC], dt.float32, tag="P2_0")
                for k in range(NJ):
                    for h, wt in enumerate(((w4h, w4l))):
                        nc.tensor.matmul(
                            Pout, wt[:, k * 10:(k + 1) * 10],
                            accns[:, k * 128:(k + 1) * 128],
                            start=(k == 0 and h == 0),
                            stop=(k == NJ - 1 and h == 1))
                nc.scalar.activation(y_sb, Pout, AF.Identity,
                                     bias=outb[:, 0:1], scale=float(-1.0 / T))
                nc.sync.dma_start(d_y[:], y_sb)

    nc.finalize()
    return nc


def kernel(**inputs):
    global _BUILT, LAST_EXEC_NS, LAST_RESULTS
    from concourse import bass_utils

    in_maps = _prep_host(inputs)
    if _BUILT is None:
        _BUILT = _build()
    nc = _BUILT

    trace = bool(int(os.environ.get("KERNEL_TRACE", "0")))
    try:
        res = bass_utils.run_bass_kernel_spmd(
            nc, in_maps, core_ids=list(range(NCORES)), trace=trace)
    except ModuleNotFoundError:
        # NTFF profile hook unavailable in this environment
        res = bass_utils.run_bass_kernel_spmd(
            nc, in_maps, core_ids=list(range(NCORES)), trace=False)
    LAST_EXEC_NS = res.exec_time_ns
    LAST_RESULTS = res

    out = np.empty((B, 10), F32)
    for c in range(NCORES):
        out[c * BC:(c + 1) * BC, :] = np.asarray(res.results[c]["y"]).T
    return out
